# revision 6
# baseline (speedup 1.0000x reference)
"""Single-head causal attention (B=4, T=4096, C=2048, H=128) on 8 TRN2 cores.

Sharding: core c handles batch b = c//2 and query parity s = c%2 — the 8
alternating 256-row query blocks of that batch.  Alternating blocks give both
parities identical causal structure after padding extents to even, so one
SPMD program serves all cores; the only rank-dependent part is the additive
logit mask, which arrives as input data.

Per-core pipeline:
  1. PE-transpose own x rows to x^T (fp32r), project Q^T/K^T/V^T (fp32r
     matmuls at full rate), transpose V^T back to natural layout.
  2. AllGather K^T and V across the 2 cores sharing the batch.
  3. For each query block: S^T = K^T.T @ Q^T per 128-key tile, additive
     causal mask on the diagonal/pad groups, exp on ScalarE (no
     max-subtraction: |logit| <= ~61 is safe in fp32), then O^T and the
     softmax denominator accumulate via PE matmuls (ones-vector trick),
     PE-transpose back and normalize.
"""

import os
import sys

for _p in ("/root/.axon_site/_ro/trn_rl_repo", "/opt/trn_rl_repo"):
    if os.path.isdir(_p) and _p not in sys.path:
        sys.path.append(_p)

import numpy as np

import concourse.bass as bass
import concourse.mybir as mybir
import concourse.tile as tile
from concourse import bacc
from concourse.bass_utils import run_bass_kernel_spmd
from concourse.masks import make_identity

f32 = mybir.dt.float32
f32r = mybir.dt.float32r
AF = mybir.ActivationFunctionType

B, T, C, H = 4, 4096, 2048, 128
NCORES = 8
QB = 256               # query-block rows (also kt unit = 2 key tiles)
NQB = T // (2 * QB)    # 8 query blocks per core
TOWN = QB * NQB        # 2048 own rows per core
NCHUNK = C // 128      # 16 contraction chunks
TB = 512               # phase-1 t-block width
SCALE = float(np.float32(C) ** np.float32(-0.05))
MASK_NEG = -1e30
GROUPS = [[0, 1], [2, 3], [4, 5], [6, 7]]

_nc_cache = {}


def _emit(nc, tc, xd, wqd, wkd, wvd, md, outd):
    import contextlib

    with contextlib.ExitStack() as stack:
        _emit_body(nc, tc, stack, xd, wqd, wkd, wvd, md, outd)


def _emit_body(nc, tc, stack, xd, wqd, wkd, wvd, md, outd):
    # ---- constants and persistent buffers ----
    const = stack.enter_context(tc.tile_pool(name="const", bufs=1))
    big = stack.enter_context(tc.tile_pool(name="big", bufs=1))
    dram = stack.enter_context(tc.tile_pool(name="dram", bufs=1, space="DRAM"))

    ident = const.tile([128, 128], f32)
    identr = const.tile([128, 128], f32r)
    make_identity(nc, ident[:])
    nc.vector.tensor_copy(identr[:], ident[:])
    ones_f = const.tile([128, 1], f32)
    onesr = const.tile([128, 1], f32r)
    nc.vector.memset(ones_f[:], 1.0)
    nc.vector.tensor_copy(onesr[:], ones_f[:])
    maskt = const.tile([128, 4, QB], f32)
    nc.sync.dma_start(out=maskt[:], in_=md[:])

    wq = const.tile([128, NCHUNK, H], f32r)
    wk = const.tile([128, NCHUNK, H], f32r)
    wv = const.tile([128, NCHUNK, H], f32r)
    for w, wd in ((wq, wqd), (wk, wkd), (wv, wvd)):
        nc.sync.dma_start(
            out=w[:], in_=wd.rearrange("(k p) h -> p k h", p=128).bitcast(f32r)
        )

    qT = big.tile([128, TOWN], f32r)            # [h, own t]
    kOwn = big.tile([128, TOWN], f32r)          # [h, own t]
    vOwn = big.tile([128, TOWN // 128, 128], f32r)  # own V tiles [tk, j, h]
    kT = big.tile([128, T], f32r)               # [h, global t]
    vSB = big.tile([128, T // 128, 128], f32r)  # global V tiles [tk, m, h]

    # ---- phase 1: transpose x, project Q/K/V ----
    with (
        tc.tile_pool(name="xn", bufs=2) as xnp,
        tc.tile_pool(name="xtp", bufs=3, space="PSUM") as xtpp,
        tc.tile_pool(name="xts", bufs=3) as xtsp,
        tc.tile_pool(name="prj", bufs=1, space="PSUM") as prj,
        tc.tile_pool(name="vnp", bufs=1, space="PSUM") as vnp,
        tc.tile_pool(name="vts", bufs=2) as vts,
    ):
        for tb in range(TOWN // TB):
            tsl = slice(tb * TB, (tb + 1) * TB)
            xn = xnp.tile([128, TB // 128, NCHUNK, 128], f32r)
            nc.sync.dma_start(
                out=xn[:],
                in_=xd[tsl, :]
                .rearrange("(j p) (k c) -> p j k c", p=128, c=128)
                .bitcast(f32r),
            )

            def do_transpose(k, xn=xn):
                xtp = xtpp.tile([128, TB], f32r, tag="xtp")
                for j in range(TB // 128):
                    nc.tensor.transpose(
                        xtp[:, j * 128 : (j + 1) * 128], xn[:, j, k, :], identr[:]
                    )
                xt = xtsp.tile([128, TB], f32r, tag="xts")
                nc.vector.tensor_copy(xt[:], xtp[:])
                return xt

            pq = prj.tile([128, TB], f32, tag="pq")
            pk = prj.tile([128, TB], f32, tag="pk")
            pv = prj.tile([128, TB], f32, tag="pv")
            xt_next = do_transpose(0)
            for k in range(NCHUNK):
                xt = xt_next
                if k + 1 < NCHUNK:
                    xt_next = do_transpose(k + 1)
                st, sp = (k == 0), (k == NCHUNK - 1)
                nc.tensor.matmul(pq[:], wq[:, k, :], xt[:], start=st, stop=sp)
                nc.tensor.matmul(pk[:], wk[:, k, :], xt[:], start=st, stop=sp)
                nc.tensor.matmul(pv[:], wv[:, k, :], xt[:], start=st, stop=sp)
            nc.scalar.copy(qT[:, tsl], pq[:])
            nc.scalar.copy(kOwn[:, tsl], pk[:])
            vt = vts.tile([128, TB], f32r, tag="vts")
            nc.scalar.copy(vt[:], pv[:])
            vn = vnp.tile([128, TB], f32r, tag="vnp")
            for j in range(TB // 128):
                nc.tensor.transpose(
                    vn[:, j * 128 : (j + 1) * 128],
                    vt[:, j * 128 : (j + 1) * 128],
                    identr[:],
                )
            nc.vector.tensor_copy(
                vOwn[:, tb * (TB // 128) : (tb + 1) * (TB // 128), :],
                vn[:].rearrange("p (j c) -> p j c", c=128),
            )

    # ---- phase 2: AllGather K^T and V within the batch pair ----
    kin = dram.tile([128, TOWN], f32)
    vin = dram.tile([128, TOWN], f32)
    kout = dram.tile([2, 128, TOWN], f32)
    vout = dram.tile([2, 128, TOWN], f32)
    nc.sync.dma_start(out=kin[:], in_=kOwn[:].bitcast(f32))
    nc.sync.dma_start(out=vin[:], in_=vOwn[:].rearrange("p j c -> p (j c)").bitcast(f32))
    nc.gpsimd.collective_compute(
        "AllGather",
        mybir.AluOpType.bypass,
        replica_groups=GROUPS,
        ins=[kin.opt()],
        outs=[kout.opt()],
    )
    nc.gpsimd.collective_compute(
        "AllGather",
        mybir.AluOpType.bypass,
        replica_groups=GROUPS,
        ins=[vin.opt()],
        outs=[vout.opt()],
    )
    for s in range(2):
        nc.sync.dma_start(
            out=kT[:].rearrange("p (i s q) -> p i s q", s=2, q=QB)[:, :, s, :],
            in_=kout[s].rearrange("p (i q) -> p i q", q=QB).bitcast(f32r),
        )
        nc.sync.dma_start(
            out=vSB[:].rearrange("p (i s u) c -> p i s u c", s=2, u=2)[:, :, s, :, :],
            in_=vout[s].rearrange("p (i u c) -> p i u c", u=2, c=128).bitcast(f32r),
        )

    # ---- phase 3: attention per query block ----
    with (
        tc.tile_pool(name="spsum", bufs=2, space="PSUM") as sp_,
        tc.tile_pool(name="opsum", bufs=1, space="PSUM") as op_,
        tc.tile_pool(name="dpsum", bufs=1, space="PSUM") as dp_,
        tc.tile_pool(name="tpsum", bufs=1, space="PSUM") as tp_,
        tc.tile_pool(name="ptp", bufs=3) as ptp,
        tc.tile_pool(name="epi", bufs=2) as epi,
    ):
        for i in range(NQB):
            G = i + 1  # groups of 4 key tiles (padded-even extent)
            qsl = slice(i * QB, (i + 1) * QB)
            ops = op_.tile([128, QB], f32, tag="opsum")
            dps = dp_.tile([1, QB], f32, tag="dpsum")

            def s_group(g, last, qsl=qsl):
                spt = sp_.tile([128, 4, QB], f32, tag="spsum")
                for j in range(4):
                    kt = 4 * g + j
                    nc.tensor.matmul(
                        spt[:, j, :],
                        kT[:, kt * 128 : (kt + 1) * 128],
                        qT[:, qsl],
                        start=True,
                        stop=True,
                    )
                if last:
                    nc.vector.tensor_add(spt[:], spt[:], maskt[:])
                ptt = ptp.tile([128, 4, QB], f32r, tag="ptp")
                nc.scalar.activation(ptt[:], spt[:], AF.Exp, scale=SCALE)
                return ptt

            ptt_next = s_group(0, last=(G == 1))
            for g in range(G):
                ptt = ptt_next
                if g + 1 < G:
                    ptt_next = s_group(g + 1, last=(g + 1 == G - 1))
                for j in range(4):
                    kt = 4 * g + j
                    st = g == 0 and j == 0
                    sp = g == G - 1 and j == 3
                    nc.tensor.matmul(
                        ops[:], vSB[:, kt, :], ptt[:, j, :], start=st, stop=sp
                    )
                    nc.tensor.matmul(
                        dps[:], onesr[:], ptt[:, j, :], start=st, stop=sp
                    )

            # epilogue: transpose O^T (PE), redistribute D across partitions
            # via tiny SBUF->SBUF DMAs, then normalize and store
            oT = epi.tile([128, QB], f32r, tag="oT")
            nc.scalar.copy(oT[:], ops[:])
            dsb = epi.tile([1, QB], f32, tag="dsb")
            nc.scalar.copy(dsb[:], dps[:])
            otp = tp_.tile([128, QB], f32r, tag="tpsum")
            for j in range(2):
                nc.tensor.transpose(
                    otp[:, j * 128 : (j + 1) * 128],
                    oT[:, j * 128 : (j + 1) * 128],
                    identr[:],
                )
            dT = epi.tile([128, 2], f32, tag="dT")
            for j in range(2):
                nc.sync.dma_start(
                    out=dT[:, j : j + 1], in_=dsb[0:1, j * 128 : (j + 1) * 128]
                )
            rp = epi.tile([128, 2], f32, tag="rp")
            nc.vector.reciprocal(rp[:], dT[:])
            outn = epi.tile([128, 2, 128], f32, tag="outn")
            for j in range(2):
                nc.vector.tensor_scalar_mul(
                    outn[:, j, :],
                    otp[:, j * 128 : (j + 1) * 128].bitcast(f32),
                    rp[:, j : j + 1],
                )
            nc.sync.dma_start(
                out=outd[qsl, :].rearrange("(j p) h -> p j h", p=128),
                in_=outn[:],
            )


def _build_nc():
    if "nc" in _nc_cache:
        return _nc_cache["nc"]
    nc = bacc.Bacc("TRN2", target_bir_lowering=False, debug=False, num_devices=NCORES)
    xd = nc.dram_tensor("x", [TOWN, C], f32, kind="ExternalInput")
    wqd = nc.dram_tensor("Wq", [C, H], f32, kind="ExternalInput")
    wkd = nc.dram_tensor("Wk", [C, H], f32, kind="ExternalInput")
    wvd = nc.dram_tensor("Wv", [C, H], f32, kind="ExternalInput")
    md = nc.dram_tensor("mask", [128, 4, QB], f32, kind="ExternalInput")
    outd = nc.dram_tensor("out", [TOWN, H], f32, kind="ExternalOutput")
    with tile.TileContext(nc) as tc:
        _emit(nc, tc, xd.ap(), wqd.ap(), wkd.ap(), wvd.ap(), md.ap(), outd.ap())
    nc.compile()
    _nc_cache["nc"] = nc
    return nc


def _make_mask(s):
    """Additive mask for the last 4 key tiles of every query block.

    Allowed iff j*128 + tk <= s*256 + tq  (j = key tile within the final
    two 256-key units; covers diagonal and even-extent padding).
    """
    m = np.zeros((128, 4, QB), dtype=np.float32)
    tk = np.arange(128)[:, None]
    tq = np.arange(QB)[None, :]
    for j in range(4):
        m[:, j, :] = np.where(j * 128 + tk <= s * 256 + tq, 0.0, MASK_NEG)
    return m


def _in_maps(x, Wq, Wk, Wv):
    x = np.ascontiguousarray(np.asarray(x, dtype=np.float32))
    Wq = np.ascontiguousarray(np.asarray(Wq, dtype=np.float32))
    Wk = np.ascontiguousarray(np.asarray(Wk, dtype=np.float32))
    Wv = np.ascontiguousarray(np.asarray(Wv, dtype=np.float32))
    masks = [_make_mask(0), _make_mask(1)]
    maps = []
    for c in range(NCORES):
        b, s = c // 2, c % 2
        xb = np.ascontiguousarray(
            x[b].reshape(NQB, 2, QB, C)[:, s].reshape(TOWN, C)
        )
        maps.append({"x": xb, "Wq": Wq, "Wk": Wk, "Wv": Wv, "mask": masks[s]})
    return maps


def _gather_out(results):
    out = np.empty((B, T, H), dtype=np.float32)
    ov = out.reshape(B, NQB, 2, QB, H)
    for c in range(NCORES):
        b, s = c // 2, c % 2
        ov[b, :, s] = results[c]["out"].reshape(NQB, QB, H)
    return out


def kernel_profiled(x, Wq, Wk, Wv):
    """Run with NTFF tracing; returns (output, exec_time_ns)."""
    try:
        import types

        import antenv

        if not hasattr(antenv, "axon_hooks"):
            from trn_agent_boot.trn_boot import _ntff_profile_via_ctypes

            mod = types.ModuleType("antenv.axon_hooks")
            mod._hook = _ntff_profile_via_ctypes("/opt/axon/libaxon_pjrt.so")
            mod.get_axon_ntff_profile_hook = lambda: mod._hook
            mod.set_axon_ntff_profile_hook = lambda h: setattr(mod, "_hook", h)
            sys.modules["antenv.axon_hooks"] = mod
            antenv.axon_hooks = mod
    except Exception:
        pass
    nc = _build_nc()
    res = run_bass_kernel_spmd(
        nc, _in_maps(x, Wq, Wk, Wv), core_ids=list(range(NCORES)), trace=True
    )
    return _gather_out(res.results), res.exec_time_ns


def kernel(x, Wq, Wk, Wv):
    nc = _build_nc()
    res = run_bass_kernel_spmd(
        nc, _in_maps(x, Wq, Wk, Wv), core_ids=list(range(NCORES))
    )
    return _gather_out(res.results)


if __name__ == "__main__":
    rng = np.random.default_rng(0)
    x = rng.standard_normal((B, T, C), dtype=np.float32)
    Wq = (rng.standard_normal((C, H)) / np.sqrt(C)).astype(np.float32)
    Wk = (rng.standard_normal((C, H)) / np.sqrt(C)).astype(np.float32)
    Wv = (rng.standard_normal((C, H)) / np.sqrt(C)).astype(np.float32)
    out = kernel(x, Wq, Wk, Wv)
    print("out", out.shape, out.dtype, float(np.abs(out).max()))


# revision 8
# speedup vs baseline: 1.2469x; 1.2469x over previous
"""Single-head causal attention (B=4, T=4096, C=2048, H=128) on 8 TRN2 cores.

Sharding: core c handles batch b = c//2 and query parity s = c%2 — the 8
alternating 256-row query blocks of that batch.  Alternating blocks give both
parities identical causal structure after padding extents to even, so one
SPMD program serves all cores; the only rank-dependent part is the additive
logit mask, which arrives as input data.

Per-core pipeline:
  1. PE-transpose own x rows to x^T (fp32r), project Q^T/K^T/V^T (fp32r
     matmuls at full rate), transpose V^T back to natural layout.
  2. AllGather K^T and V across the 2 cores sharing the batch.
  3. For each query block: S^T = K^T.T @ Q^T per 128-key tile, additive
     causal mask on the diagonal/pad groups, exp on ScalarE (no
     max-subtraction: |logit| <= ~61 is safe in fp32), then O^T and the
     softmax denominator accumulate via PE matmuls (ones-vector trick),
     PE-transpose back and normalize.
"""

import os
import sys

for _p in ("/root/.axon_site/_ro/trn_rl_repo", "/opt/trn_rl_repo"):
    if os.path.isdir(_p) and _p not in sys.path:
        sys.path.append(_p)

import numpy as np

import concourse.bass as bass
import concourse.mybir as mybir
import concourse.tile as tile
from concourse import bacc
from concourse.bass_utils import run_bass_kernel_spmd
from concourse.masks import make_identity

f32 = mybir.dt.float32
f32r = mybir.dt.float32r
AF = mybir.ActivationFunctionType

B, T, C, H = 4, 4096, 2048, 128
NCORES = 8
QB = 512               # query-block rows (= causal pad unit = 4 key tiles)
NQB = T // (2 * QB)    # 8 query blocks per core
TOWN = QB * NQB        # 2048 own rows per core
NCHUNK = C // 128      # 16 contraction chunks
TB = 512               # phase-1 t-block width
SCALE = float(np.float32(C) ** np.float32(-0.05))
MASK_NEG = -1e30
GROUPS = [[0, 1], [2, 3], [4, 5], [6, 7]]

_nc_cache = {}


def _emit(nc, tc, xd, wqd, wkd, wvd, md, outd):
    import contextlib

    with contextlib.ExitStack() as stack:
        _emit_body(nc, tc, stack, xd, wqd, wkd, wvd, md, outd)


def _emit_body(nc, tc, stack, xd, wqd, wkd, wvd, md, outd):
    # ---- constants and persistent buffers ----
    const = stack.enter_context(tc.tile_pool(name="const", bufs=1))
    big = stack.enter_context(tc.tile_pool(name="big", bufs=1))
    dram = stack.enter_context(tc.tile_pool(name="dram", bufs=1, space="DRAM"))

    ident = const.tile([128, 128], f32)
    identr = const.tile([128, 128], f32r)
    make_identity(nc, ident[:])
    nc.vector.tensor_copy(identr[:], ident[:])
    ones_f = const.tile([128, 1], f32)
    onesr = const.tile([128, 1], f32r)
    nc.vector.memset(ones_f[:], 1.0)
    nc.vector.tensor_copy(onesr[:], ones_f[:])
    maskt = const.tile([128, 8, QB], f32)
    nc.sync.dma_start(out=maskt[:], in_=md[:])

    wq = const.tile([128, NCHUNK, H], f32r)
    wk = const.tile([128, NCHUNK, H], f32r)
    wv = const.tile([128, NCHUNK, H], f32r)
    for w, wd in ((wq, wqd), (wk, wkd), (wv, wvd)):
        nc.sync.dma_start(
            out=w[:], in_=wd.rearrange("(k p) h -> p k h", p=128).bitcast(f32r)
        )

    qT = big.tile([128, TOWN], f32r)            # [h, own t]
    kOwn = big.tile([128, TOWN], f32r)          # [h, own t]
    vOwn = big.tile([128, TOWN // 128, 128], f32r)  # own V tiles [tk, j, h]
    kT = big.tile([128, T], f32r)               # [h, global t]
    vSB = big.tile([128, T // 128, 128], f32r)  # global V tiles [tk, m, h]

    # ---- phase 1: transpose x, project Q/K/V ----
    with (
        tc.tile_pool(name="xn", bufs=2) as xnp,
        tc.tile_pool(name="xtp", bufs=3, space="PSUM") as xtpp,
        tc.tile_pool(name="xts", bufs=3) as xtsp,
        tc.tile_pool(name="prj", bufs=1, space="PSUM") as prj,
        tc.tile_pool(name="vnp", bufs=1, space="PSUM") as vnp,
        tc.tile_pool(name="vts", bufs=2) as vts,
    ):
        for tb in range(TOWN // TB):
            tsl = slice(tb * TB, (tb + 1) * TB)
            xn = xnp.tile([128, TB // 128, NCHUNK, 128], f32r)
            for j in range(TB // 128):
                nc.sync.dma_start(
                    out=xn[:, j, :, :],
                    in_=xd[tb * TB + j * 128 : tb * TB + (j + 1) * 128, :]
                    .rearrange("p (k c) -> p k c", c=128)
                    .bitcast(f32r),
                )

            def do_transpose(k, xn=xn):
                xtp = xtpp.tile([128, TB], f32r, tag="xtp")
                for j in range(TB // 128):
                    nc.tensor.transpose(
                        xtp[:, j * 128 : (j + 1) * 128], xn[:, j, k, :], identr[:]
                    )
                xt = xtsp.tile([128, TB], f32r, tag="xts")
                nc.vector.tensor_copy(xt[:], xtp[:])
                return xt

            pq = prj.tile([128, TB], f32, tag="pq")
            pk = prj.tile([128, TB], f32, tag="pk")
            pv = prj.tile([128, TB], f32, tag="pv")
            xt_next = do_transpose(0)
            for k in range(NCHUNK):
                xt = xt_next
                if k + 1 < NCHUNK:
                    xt_next = do_transpose(k + 1)
                st, sp = (k == 0), (k == NCHUNK - 1)
                nc.tensor.matmul(pq[:], wq[:, k, :], xt[:], start=st, stop=sp)
                nc.tensor.matmul(pk[:], wk[:, k, :], xt[:], start=st, stop=sp)
                nc.tensor.matmul(pv[:], wv[:, k, :], xt[:], start=st, stop=sp)
            nc.scalar.copy(qT[:, tsl], pq[:])
            nc.scalar.copy(kOwn[:, tsl], pk[:])
            vt = vts.tile([128, TB], f32r, tag="vts")
            nc.scalar.copy(vt[:], pv[:])
            vn = vnp.tile([128, TB], f32r, tag="vnp")
            for j in range(TB // 128):
                nc.tensor.transpose(
                    vn[:, j * 128 : (j + 1) * 128],
                    vt[:, j * 128 : (j + 1) * 128],
                    identr[:],
                )
            nc.vector.tensor_copy(
                vOwn[:, tb * (TB // 128) : (tb + 1) * (TB // 128), :],
                vn[:].rearrange("p (j c) -> p j c", c=128),
            )

    # ---- phase 2: AllGather K^T and V within the batch pair ----
    # Two half-gathers: half h covers own rows [h*1024, (h+1)*1024) which is
    # exactly global keys [h*2048, (h+1)*2048) for both parities, so half A
    # overlaps phase-1 tail and half B overlaps the first attention blocks.
    HW_ = TOWN // 2  # 1024 own rows per gather half
    for gh in range(2):
        osl = slice(gh * HW_, (gh + 1) * HW_)
        kin = dram.tile([128, HW_], f32, tag=f"kin{gh}")
        vin = dram.tile([128, HW_], f32, tag=f"vin{gh}")
        kout = dram.tile([2, 128, HW_], f32, tag=f"kout{gh}")
        vout = dram.tile([2, 128, HW_], f32, tag=f"vout{gh}")
        nc.sync.dma_start(out=kin[:], in_=kOwn[:, osl].bitcast(f32))
        nc.sync.dma_start(
            out=vin[:],
            in_=vOwn[:, gh * (HW_ // 128) : (gh + 1) * (HW_ // 128), :]
            .rearrange("p j c -> p (j c)")
            .bitcast(f32),
        )
        nc.gpsimd.collective_compute(
            "AllGather",
            mybir.AluOpType.bypass,
            replica_groups=GROUPS,
            ins=[kin.opt()],
            outs=[kout.opt()],
        )
        nc.gpsimd.collective_compute(
            "AllGather",
            mybir.AluOpType.bypass,
            replica_groups=GROUPS,
            ins=[vin.opt()],
            outs=[vout.opt()],
        )
        gsl = slice(gh * 2 * HW_, (gh + 1) * 2 * HW_)
        for s in range(2):
            nc.sync.dma_start(
                out=kT[:, gsl].rearrange("p (i s q) -> p i s q", s=2, q=QB)[:, :, s, :],
                in_=kout[s].rearrange("p (i q) -> p i q", q=QB).bitcast(f32r),
            )
            nc.sync.dma_start(
                out=vSB[:, gh * 16 : (gh + 1) * 16, :].rearrange(
                    "p (i s u) c -> p i s u c", s=2, u=4
                )[:, :, s, :, :],
                in_=vout[s].rearrange("p (i u c) -> p i u c", u=4, c=128).bitcast(f32r),
            )

    # ---- phase 3: attention per query block (QB=512) ----
    # Block i runs 4i+4 key tiles (padded-even extent, identical both
    # parities) in groups of 2 tiles; the last 4 groups (= last 2 pad units)
    # get the additive causal/pad mask before exp.
    with (
        tc.tile_pool(name="spsum", bufs=2, space="PSUM") as sp_,
        tc.tile_pool(name="opsum", bufs=2, space="PSUM") as op_,
        tc.tile_pool(name="dpsum", bufs=1, space="PSUM") as dp_,
        tc.tile_pool(name="tpsum", bufs=1, space="PSUM") as tp_,
        tc.tile_pool(name="ptp", bufs=3) as ptp,
        tc.tile_pool(name="epi", bufs=2) as epi,
    ):
        for i in range(NQB):
            NG = 4 * i + 4  # groups of 2 key tiles
            NKT = 2 * NG
            qsl = slice(i * QB, (i + 1) * QB)
            ops = op_.tile([128, QB], f32, tag="opsum")
            dps = dp_.tile([1, QB], f32, tag="dpsum")

            def s_group(g, NG=NG, qsl=qsl):
                spt = sp_.tile([128, 2, QB], f32, tag="spsum")
                for j in range(2):
                    kt = 2 * g + j
                    nc.tensor.matmul(
                        spt[:, j, :],
                        kT[:, kt * 128 : (kt + 1) * 128],
                        qT[:, qsl],
                        start=True,
                        stop=True,
                    )
                e = NG - 1 - g  # groups from the end
                if e < 4:
                    msl = maskt[:, 6 - 2 * e : 8 - 2 * e, :]
                    nc.vector.tensor_add(spt[:], spt[:], msl)
                ptt = ptp.tile([128, 2, QB], f32r, tag="ptp")
                nc.scalar.activation(ptt[:], spt[:], AF.Exp, scale=SCALE)
                return ptt

            ptt_next = s_group(0)
            for g in range(NG):
                ptt = ptt_next
                if g + 1 < NG:
                    ptt_next = s_group(g + 1)
                for j in range(2):
                    kt = 2 * g + j
                    st = kt == 0
                    sp = kt == NKT - 1
                    nc.tensor.matmul(
                        ops[:], vSB[:, kt, :], ptt[:, j, :], start=st, stop=sp
                    )
                    nc.tensor.matmul(
                        dps[:], onesr[:], ptt[:, j, :], start=st, stop=sp
                    )

            # epilogue: transpose O^T (PE), redistribute D across partitions
            # via tiny SBUF->SBUF DMAs, then normalize and store
            oT = epi.tile([128, QB], f32r, tag="oT")
            nc.vector.tensor_copy(oT[:], ops[:].bitcast(f32r))
            dsb = epi.tile([1, QB], f32, tag="dsb")
            nc.vector.tensor_copy(dsb[:], dps[:])
            otp = tp_.tile([128, QB], f32r, tag="tpsum")
            for j in range(QB // 128):
                nc.tensor.transpose(
                    otp[:, j * 128 : (j + 1) * 128],
                    oT[:, j * 128 : (j + 1) * 128],
                    identr[:],
                )
            dT = epi.tile([128, QB // 128], f32, tag="dT")
            for j in range(QB // 128):
                nc.sync.dma_start(
                    out=dT[:, j : j + 1], in_=dsb[0:1, j * 128 : (j + 1) * 128]
                )
            rp = epi.tile([128, QB // 128], f32, tag="rp")
            nc.vector.reciprocal(rp[:], dT[:])
            outn = epi.tile([128, QB // 128, 128], f32, tag="outn")
            for j in range(QB // 128):
                nc.vector.tensor_scalar_mul(
                    outn[:, j, :],
                    otp[:, j * 128 : (j + 1) * 128].bitcast(f32),
                    rp[:, j : j + 1],
                )
            nc.sync.dma_start(
                out=outd[qsl, :].rearrange("(j p) h -> p j h", p=128),
                in_=outn[:],
            )


def _build_nc():
    if "nc" in _nc_cache:
        return _nc_cache["nc"]
    nc = bacc.Bacc("TRN2", target_bir_lowering=False, debug=False, num_devices=NCORES)
    xd = nc.dram_tensor("x", [TOWN, C], f32, kind="ExternalInput")
    wqd = nc.dram_tensor("Wq", [C, H], f32, kind="ExternalInput")
    wkd = nc.dram_tensor("Wk", [C, H], f32, kind="ExternalInput")
    wvd = nc.dram_tensor("Wv", [C, H], f32, kind="ExternalInput")
    md = nc.dram_tensor("mask", [128, 8, QB], f32, kind="ExternalInput")
    outd = nc.dram_tensor("out", [TOWN, H], f32, kind="ExternalOutput")
    with tile.TileContext(nc) as tc:
        _emit(nc, tc, xd.ap(), wqd.ap(), wkd.ap(), wvd.ap(), md.ap(), outd.ap())
    nc.compile()
    _nc_cache["nc"] = nc
    return nc


def _make_mask(s):
    """Additive mask for the last 8 key tiles of every query block.

    Allowed iff j*128 + tk <= s*QB + tq  (j = key tile within the final two
    QB-key units; covers the causal diagonal and the even-extent padding).
    """
    m = np.zeros((128, 8, QB), dtype=np.float32)
    tk = np.arange(128)[:, None]
    tq = np.arange(QB)[None, :]
    for j in range(8):
        m[:, j, :] = np.where(j * 128 + tk <= s * QB + tq, 0.0, MASK_NEG)
    return m


def _in_maps(x, Wq, Wk, Wv):
    x = np.ascontiguousarray(np.asarray(x, dtype=np.float32))
    Wq = np.ascontiguousarray(np.asarray(Wq, dtype=np.float32))
    Wk = np.ascontiguousarray(np.asarray(Wk, dtype=np.float32))
    Wv = np.ascontiguousarray(np.asarray(Wv, dtype=np.float32))
    masks = [_make_mask(0), _make_mask(1)]
    maps = []
    for c in range(NCORES):
        b, s = c // 2, c % 2
        xb = np.ascontiguousarray(
            x[b].reshape(NQB, 2, QB, C)[:, s].reshape(TOWN, C)
        )
        maps.append({"x": xb, "Wq": Wq, "Wk": Wk, "Wv": Wv, "mask": masks[s]})
    return maps


def _gather_out(results):
    out = np.empty((B, T, H), dtype=np.float32)
    ov = out.reshape(B, NQB, 2, QB, H)
    for c in range(NCORES):
        b, s = c // 2, c % 2
        ov[b, :, s] = results[c]["out"].reshape(NQB, QB, H)
    return out


def kernel_profiled(x, Wq, Wk, Wv):
    """Run with NTFF tracing; returns (output, exec_time_ns)."""
    try:
        import types

        import antenv

        if not hasattr(antenv, "axon_hooks"):
            from trn_agent_boot.trn_boot import _ntff_profile_via_ctypes

            mod = types.ModuleType("antenv.axon_hooks")
            mod._hook = _ntff_profile_via_ctypes("/opt/axon/libaxon_pjrt.so")
            mod.get_axon_ntff_profile_hook = lambda: mod._hook
            mod.set_axon_ntff_profile_hook = lambda h: setattr(mod, "_hook", h)
            sys.modules["antenv.axon_hooks"] = mod
            antenv.axon_hooks = mod
    except Exception:
        pass
    nc = _build_nc()
    res = run_bass_kernel_spmd(
        nc, _in_maps(x, Wq, Wk, Wv), core_ids=list(range(NCORES)), trace=True
    )
    return _gather_out(res.results), res.exec_time_ns


def kernel(x, Wq, Wk, Wv):
    nc = _build_nc()
    res = run_bass_kernel_spmd(
        nc, _in_maps(x, Wq, Wk, Wv), core_ids=list(range(NCORES))
    )
    return _gather_out(res.results)


if __name__ == "__main__":
    rng = np.random.default_rng(0)
    x = rng.standard_normal((B, T, C), dtype=np.float32)
    Wq = (rng.standard_normal((C, H)) / np.sqrt(C)).astype(np.float32)
    Wk = (rng.standard_normal((C, H)) / np.sqrt(C)).astype(np.float32)
    Wv = (rng.standard_normal((C, H)) / np.sqrt(C)).astype(np.float32)
    out = kernel(x, Wq, Wk, Wv)
    print("out", out.shape, out.dtype, float(np.abs(out).max()))
